# revision 18
# baseline (speedup 1.0000x reference)
"""TRN2 Bass kernel: causal multi-head self-attention block (QKV proj ->
causal softmax attention -> output proj) for B=4, T=2048, C=1024, H=16.

Sharding over 8 NeuronCores: core c handles batch b = c//2 and head-group
g = c%2 (8 of the 16 heads, i.e. 512 of the 1024 hidden channels).  Each
core computes its batch's QKV projection restricted to its head-group's
columns, causal attention for its 8 heads, and a *partial* output
projection (its 512 rows of W_proj).  The host sums the two partial
outputs per batch and adds the host-foldable bias terms
(b_proj, and b_qkv's V part folded through W_proj).

Per-core device kernel (bf16 matmuls, fp32 PSUM accumulation):
  - x arrives pre-transposed (xT, [C,T]) so the QKV projection needs no
    on-device transpose; Q^T/K^T are produced in [d_head, T] layout,
    which is exactly the scores-matmul operand layout (contraction over
    d on the partition axis): S^T blocks [128 k, 512 q] are one matmul
    each, with causal block skipping and diagonal blocks shortened to
    their live [delta:512] q-range.
  - exp() runs on the Scalar engine straight out of PSUM (the 1/sqrt(64)
    scale folded in); adjacent full blocks share a 2-bank PSUM tile and
    a single exp; diagonal blocks are causal-masked after the exp by a
    0/1 bf16 strip multiply (2x DVE mode).
  - PV runs *flipped*: out[q,d] tiles [128, 65] with lhsT = est (the
    exp'd scores block, q-sub-sliced) and rhs = V (with a trailing
    all-ones column per head so the softmax denominator accumulates in
    column 64 for free).  A flipped-PV matmul costs 65 PE rows instead
    of the 512 an [65, 512] block costs, and the denominator lands
    per-partition, so normalization is one batched reciprocal plus
    native tensor_scalar multiplies -- no partition broadcast needed.
  - The normalized attn-out staging tiles [q, ch] are moved into the
    [ch, q] layout the output projection needs by a single XBAR
    DMA-transpose per head-pair and strip (SBUF->SBUF, no PE/DVE work);
    the very last pair instead uses PE transposes + a DVE copy to keep
    the tail off the long DMA-transpose latency.
  - Scheduling: the score/exp stream is software-pipelined ahead of the
    PV matmuls, and QKV-strip / output-projection work is threaded
    through the attention rounds as PE filler (weighted toward the
    later, exp-heavy rounds) so the Tensor engine stays busy while the
    Scalar engine works through the exp stream.
  - DMA: weights/activations load as one descriptor-batched DMA each
    (HWDGE issue overhead dominates many-small-DMA plans); the strip-0
    x and W_q transfers are split in halves so the first projection
    matmuls can start as early as possible.

Toolchain workarounds (this container's walrus build): instructions may
carry at most one sync wait, so excess waits are moved onto InstNoOp
wait-carriers (_split_sync_waits), including on the TileContext tail
drain (_SplitDrainTileContext).
"""

import numpy as np
import ml_dtypes

import concourse.bass as bass
import concourse.mybir as mybir
import concourse.tile as tile
from concourse.bass_utils import run_bass_kernel_spmd
from concourse.vector_clock import ScopedClock

# ---------------------------------------------------------------- problem dims
B = 4
T = 2048
C = 1024
H = 16
DH = 64           # head dim
NCORES = 8
HL = H // 2       # heads per core (head-group of 8)
CL = HL * DH      # 512 local channels per core

F32 = mybir.dt.float32
BF16 = mybir.dt.bfloat16
AF = mybir.ActivationFunctionType
SCALE = 1.0 / np.sqrt(DH)

PVW = 68          # per-q-sub accumulator stride in the pso tile (65 used)


class _SplitDrainTileContext(tile.TileContext):
    """TileContext whose tail drain splits its semaphore waits across
    multiple drain instructions; the walrus build in this container
    rejects CTRL instructions carrying more than ~2 sync waits."""

    MAX_WAITS = 1

    def _drain_and_barrier(self, tick_clock, wait_clock):
        nc = self.nc
        drain_inst = nc.sync.drain()
        wait_clock.add_sem_waits(
            drain_inst.ins, ScopedClock({None: tick_clock.global_clock})
        )
        si = drain_inst.ins.sync_info
        waits = list(si.on_wait or []) if si else []
        if len(waits) > self.MAX_WAITS:
            drain_inst.ins.sync_info = mybir.SyncInfo(
                on_wait=waits[: self.MAX_WAITS],
                on_update=list(si.on_update or []),
            )
            rest = waits[self.MAX_WAITS:]
            for i in range(0, len(rest), self.MAX_WAITS):
                extra = nc.sync.drain()
                extra.ins.sync_info = mybir.SyncInfo(
                    on_wait=rest[i : i + self.MAX_WAITS], on_update=[]
                )
        nc.all_engine_barrier()
        assert self.sems is not None
        popped = nc._tile_sem_poison_stack.pop()
        assert popped is self._sem_poison
        nc.clear_and_free_semaphores(list(self.sems.allocated().values()))
        nc.all_engine_barrier()


def _persist(pp, shape, dtype, name):
    return pp.tile(shape, dtype, name=name, tag=name)


_MAX_WAITS = 1


def _split_sync_waits(nc):
    """The walrus build here accepts only a small number of sync waits per
    instruction.  Move excess waits onto InstNoOp wait-carriers inserted
    just before the over-subscribed instruction on the same engine."""
    for bb in nc.main_func.blocks:
        new_insts = []
        changed = False
        for ins in bb.instructions:
            si = ins.sync_info
            waits = list(si.on_wait or []) if si else []
            if len(waits) > _MAX_WAITS:
                changed = True
                extra, keep = waits[_MAX_WAITS:], waits[: _MAX_WAITS]
                for i in range(0, len(extra), _MAX_WAITS):
                    nop = mybir.InstNoOp(name=f"I-waitsplit-{nc.next_id()}")
                    nop.engine = ins.engine
                    nop.sync_info = mybir.SyncInfo(
                        on_wait=extra[i : i + _MAX_WAITS], on_update=[]
                    )
                    new_insts.append(nop)
                ins.sync_info = mybir.SyncInfo(
                    on_wait=keep, on_update=list(si.on_update or [])
                )
            new_insts.append(ins)
        if changed:
            bb.instructions = new_insts
    return nc


def build_program(loop_reps: int = 1):
    """Build the per-core program.  loop_reps > 1 unrolls the whole body
    that many times in one NEFF (used only for wall-clock timing; the
    grading path uses 1)."""
    nc = bass.Bass(trn_type="TRN2")

    xt = nc.dram_tensor("xt", [T // 512, C, 512], BF16, kind="ExternalInput")
    wq = nc.dram_tensor("wq", [C, CL], BF16, kind="ExternalInput")
    wk = nc.dram_tensor("wk", [C, CL], BF16, kind="ExternalInput")
    wv = nc.dram_tensor("wv", [C, CL], BF16, kind="ExternalInput")
    wp = nc.dram_tensor("wp", [CL, C], BF16, kind="ExternalInput")
    bq = nc.dram_tensor("bq", [128, 4], F32, kind="ExternalInput")
    bk = nc.dram_tensor("bk", [128, 4], F32, kind="ExternalInput")
    mska = nc.dram_tensor("mska", [128, 896], BF16, kind="ExternalInput")
    mskb = nc.dram_tensor("mskb", [128, 384], BF16, kind="ExternalInput")
    ident = nc.dram_tensor("ident", [128, 128], BF16, kind="ExternalInput")
    out = nc.dram_tensor("out", [T, C], BF16, kind="ExternalOutput")

    NCC = C // 128            # 8 c-chunks of the model dim
    NTC = T // 128            # 16 t-chunks

    with _SplitDrainTileContext(nc) as tc, tc.tile_pool(
        name="persist", bufs=1
    ) as pp:
        # ------------------------------------------------ persistent SBUF
        xt_sb = _persist(pp, [128, NCC, T], BF16, "xts")
        wq_sb = _persist(pp, [128, NCC, CL], BF16, "wqs")
        wk_sb = _persist(pp, [128, NCC, CL], BF16, "wks")
        wv_sb = _persist(pp, [128, NCC, CL], BF16, "wvs")
        wp_sb = _persist(pp, [128, CL // 128, C], BF16, "wps")
        bq_sb = _persist(pp, [128, 4], F32, "bqs")
        bk_sb = _persist(pp, [128, 4], F32, "bks")
        mska_sb = _persist(pp, [128, 896], BF16, "mskas")
        mskb_sb = _persist(pp, [128, 384], BF16, "mskbs")
        ident_sb = _persist(pp, [128, 128], BF16, "idents")
        wdum_sb = _persist(pp, [128, 512], BF16, "wdums")
        qt_sb = [_persist(pp, [128, T], BF16, f"qts{p}") for p in range(4)]
        kt_sb = [_persist(pp, [128, T], BF16, f"kts{p}") for p in range(4)]
        # V with a trailing ones column per head: [t-part, t-chunk, head, 64+1]
        v_sb = _persist(pp, [128, NTC, HL, DH + 1], BF16, "vsb")
        aot_sb = [
            [_persist(pp, [128, 512], BF16, f"aots{p}_{jj}") for jj in range(T // 512)]
            for p in range(4)
        ]

        for _rep in range(loop_reps):
            _emit_body(
                nc, tc, xt, wq, wk, wv, wp, bq, bk, mska, mskb, ident, out,
                xt_sb, wq_sb, wk_sb, wv_sb, wp_sb, bq_sb, bk_sb, mska_sb,
                mskb_sb, ident_sb, wdum_sb, qt_sb, kt_sb, v_sb, aot_sb,
            )
    _split_sync_waits(nc)
    return nc


def _emit_body(
    nc, tc, xt, wq, wk, wv, wp, bq, bk, mska, mskb, ident, out,
    xt_sb, wq_sb, wk_sb, wv_sb, wp_sb, bq_sb, bk_sb, mska_sb,
    mskb_sb, ident_sb, wdum_sb, qt_sb, kt_sb, v_sb, aot_sb,
):
    NCC = C // 128
    NTS = T // 512

    def _rows(w, lo, hi):
        # w[lo:hi, :] laid out [p, chunk, outcol]
        return w[lo:hi].rearrange("(c p) o -> p c o", p=128)

    # All large transfers on the sync HWDGE queue in exact need-order, so
    # the (serial) DMA-engine track processes them in the order the PE
    # stream consumes them.  Tiny transfers ride the scalar queue.
    nc.sync.dma_start(
        out=xt_sb[:, 0:4, 0:512], in_=xt[0, 0:512].rearrange("(c p) t -> p c t", p=128)
    )
    nc.sync.dma_start(
        out=wq_sb[:, :, 0:256],
        in_=wq[:, 0:256].rearrange("(c p) o -> p c o", p=128),
    )
    nc.sync.dma_start(
        out=xt_sb[:, 4:8, 0:512],
        in_=xt[0, 512:1024].rearrange("(c p) t -> p c t", p=128),
    )
    nc.sync.dma_start(
        out=wq_sb[:, :, 256:512],
        in_=wq[:, 256:512].rearrange("(c p) o -> p c o", p=128),
    )
    nc.sync.dma_start(out=wk_sb, in_=_rows(wk, 0, C))
    nc.sync.dma_start(out=wv_sb, in_=_rows(wv, 0, C))
    for j in range(1, NTS):
        nc.sync.dma_start(
            out=xt_sb[:, :, 512 * j : 512 * j + 512],
            in_=xt[j].rearrange("(c p) t -> p c t", p=128),
        )
    nc.sync.dma_start(out=wp_sb, in_=wp.rearrange("(c p) o -> p c o", p=128))
    nc.scalar.dma_start(out=bq_sb, in_=bq[:])
    nc.scalar.dma_start(out=bk_sb, in_=bk[:])
    nc.scalar.dma_start(out=mska_sb, in_=mska[:])
    nc.scalar.dma_start(out=mskb_sb, in_=mskb[:])
    nc.scalar.dma_start(out=ident_sb, in_=ident[:])
    nc.vector.memset(wdum_sb, 0.0)
    nc.vector.memset(v_sb[:, :, :, DH : DH + 1], 1.0)

    # ------------------------------------------------ pools
    with (
        tc.tile_pool(name="pmm", bufs=2, space="PSUM") as pmm,
        tc.tile_pool(name="pst", bufs=2, space="PSUM") as pstp,
        tc.tile_pool(name="pso", bufs=2, space="PSUM") as psop,
        tc.tile_pool(name="pest", bufs=10) as pest,
        tc.tile_pool(name="pstg", bufs=3) as pstg,
        tc.tile_pool(name="prec", bufs=4) as prec,
        tc.tile_pool(name="pout", bufs=3) as pout,
    ):

        def qk_strip_units(j):
            """Q^T/K^T projection for t-strip j, yielded in PE-sized
            units so attention rounds can interleave them as filler.
            Strip 0 runs all Q before all K so the first matmuls only
            depend on the wq/xt transfers."""
            t0 = 512 * j
            plan = (
                [(p, 0) for p in range(4)] + [(p, 1) for p in range(4)]
                if j == 0
                else [(p, qk) for p in range(4) for qk in range(2)]
            )
            for p, qk in plan:
                w_sb, b_sb, o_sb = (
                    (wq_sb, bq_sb, qt_sb) if qk == 0 else (wk_sb, bk_sb, kt_sb)
                )
                ps = pmm.tile([128, 512], F32, name="psqk", tag="mm")
                for cc in range(NCC):
                    nc.tensor.matmul(
                        ps,
                        lhsT=w_sb[:, cc, 128 * p : 128 * p + 128],
                        rhs=xt_sb[:, cc, t0 : t0 + 512],
                        start=(cc == 0),
                        stop=(cc == NCC - 1),
                    )
                nc.vector.tensor_scalar_add(
                    o_sb[p][:, t0 : t0 + 512], ps, b_sb[:, p : p + 1]
                )
                yield

        def v_strip_units(j):
            """V projection for the 4 t-chunks of strip j."""
            for ic in range(4 * j, 4 * j + 4):
                psv = pmm.tile([128, 512], F32, name="psv", tag="mm")
                for cc in range(NCC):
                    nc.tensor.matmul(
                        psv,
                        lhsT=xt_sb[:, cc, 128 * ic : 128 * ic + 128],
                        rhs=wv_sb[:, cc, :],
                        start=(cc == 0),
                        stop=(cc == NCC - 1),
                    )
                nc.vector.tensor_copy(
                    v_sb[:, ic, :, 0:DH],
                    psv.rearrange("p (h d) -> p h d", h=HL),
                )
                yield

        def qkv_strip_units(j):
            yield from qk_strip_units(j)
            yield from v_strip_units(j)

        def _proj_mm(j, qi, nh, ccs):
            ps3 = pmm.tile([128, 512], F32, name="ps3", tag="mm")
            for ci, cc in enumerate(ccs):
                nc.tensor.matmul(
                    ps3,
                    lhsT=aot_sb[cc][j][:, 128 * (qi % 4) : 128 * (qi % 4) + 128],
                    rhs=wp_sb[:, cc, 512 * nh : 512 * nh + 512],
                    start=(ci == 0),
                    stop=(ci == len(ccs) - 1),
                )
            return ps3

        def proj_units(j):
            """Partial output projection for the 4 t-chunks of strip j."""
            for qi in range(4 * j, 4 * j + 4):
                ostg = pout.tile([128, C], BF16, name="ostg", tag="ostg")
                for nh in range(2):
                    ps3 = _proj_mm(j, qi, nh, range(CL // 128))
                    nc.vector.tensor_copy(ostg[:, 512 * nh : 512 * nh + 512], ps3)
                    nc.sync.dma_start(
                        out=out[128 * qi : 128 * qi + 128, 512 * nh : 512 * nh + 512],
                        in_=ostg[:, 512 * nh : 512 * nh + 512],
                    )
                    yield

        lstg = {}  # (qi, nh) -> bf16 staging of the cc{0,1} half-sum

        def proj_last_part1(j):
            """First half-accumulation (pairs 0,1) of the last strip's
            projection -- runs as round-3 filler once aot[0..1][j] exist,
            adding PE work to the exp-bound closing round."""
            for qi in range(4 * j, 4 * j + 4):
                for nh in range(2):
                    ps3 = _proj_mm(j, qi, nh, (0, 1))
                    st_ = pout.tile([128, 512], BF16, name="lstg", tag="lstg", bufs=16)
                    nc.vector.tensor_copy(st_, ps3)
                    lstg[(qi, nh)] = st_
                    yield

        def proj_last_part2(j):
            """Second half (pairs 2,3) + combine + store; the two column
            halves share one output DMA (HWDGE issue time gates the tail)."""
            for qi in range(4 * j, 4 * j + 4):
                ostg = pout.tile([128, C], BF16, name="ostg", tag="ostg")
                for nh in range(2):
                    ps3 = _proj_mm(j, qi, nh, (2, 3))
                    nc.vector.tensor_add(
                        ostg[:, 512 * nh : 512 * nh + 512], ps3, lstg[(qi, nh)]
                    )
                    if nh == 1:
                        nc.sync.dma_start(
                            out=out[128 * qi : 128 * qi + 128, :], in_=ostg
                        )
                    yield

        PV_LAG = 4  # plan steps (2 k-chunks each) the PV trails behind

        stages = {}  # pair -> stage tile for the pair currently in flight

        def attn(h, j, pump, tail=False):
            """Causal attention for head h over q-strip j.  The score/exp
            stream is software-pipelined PV_LAG plan-steps (2 k-chunks
            each) ahead of the flipped PV accumulation; the four
            diagonal blocks are packed into two PSUM tiles so they cost
            two exp/mask instructions instead of four; `pump` is called
            once per step to emit filler projection work that keeps PE
            busy while the Scalar engine works through the exp stream."""
            pt, off = h // 2, (h % 2) * DH
            qt_h = qt_sb[pt][off : off + DH, :]
            kt_h = kt_sb[pt][off : off + DH, :]
            q0 = 512 * j
            nk = 4 * (j + 1)
            pso = psop.tile([128, 4, PVW], F32, name="pso", tag="pso")
            # ests[ip] = (ap, shift): PV lhsT for q-sub n is
            # ap[:, 128n - shift : 128n - shift + 128]
            ests = [None] * nk
            nfull = 4 * j

            def emit_full_pair(i):
                pst2 = pstp.tile([128, 1024], F32, name="pst2", tag="st")
                for half in range(2):
                    nc.tensor.matmul(
                        pst2[:, 512 * half : 512 * half + 512],
                        lhsT=kt_h[:, 128 * (i + half) : 128 * (i + half) + 128],
                        rhs=qt_h[:, q0 : q0 + 512],
                        start=True,
                        stop=True,
                    )
                est2 = pest.tile([128, 1024], BF16, name="est2", tag="est")
                nc.scalar.activation(est2, pst2, AF.Exp, scale=SCALE)
                ests[i] = (est2[:, 0:512], 0)
                ests[i + 1] = (est2[:, 512:1024], 0)

            def emit_diag_pair(pos):
                # pos 0: diagonals m=0,1 packed at cols [0:512 | 512:896]
                # pos 1: diagonals m=2,3 packed at cols [0:256 | 256:384]
                m0 = 2 * pos
                w0 = 512 - 128 * m0          # live width of diag m0
                w1 = w0 - 128                # live width of diag m0+1
                pst2 = pstp.tile([128, 1024], F32, name="pst2", tag="st")
                nc.tensor.matmul(
                    pst2[:, 0:w0],
                    lhsT=kt_h[:, 128 * (nfull + m0) : 128 * (nfull + m0) + 128],
                    rhs=qt_h[:, q0 + 128 * m0 : q0 + 512],
                    start=True,
                    stop=True,
                )
                nc.tensor.matmul(
                    pst2[:, w0 : w0 + w1],
                    lhsT=kt_h[:, 128 * (nfull + m0 + 1) : 128 * (nfull + m0 + 1) + 128],
                    rhs=qt_h[:, q0 + 128 * (m0 + 1) : q0 + 512],
                    start=True,
                    stop=True,
                )
                est2 = pest.tile([128, 1024], BF16, name="est2", tag="est")
                nc.scalar.activation(
                    est2[:, 0 : w0 + w1], pst2[:, 0 : w0 + w1], AF.Exp, scale=SCALE
                )
                mk = mska_sb if pos == 0 else mskb_sb
                nc.vector.tensor_mul(
                    est2[:, 0 : w0 + w1], est2[:, 0 : w0 + w1], mk[:, 0 : w0 + w1]
                )
                ests[nfull + m0] = (est2[:, 0:w0], 128 * m0)
                ests[nfull + m0 + 1] = (est2[:, w0 : w0 + w1], 128 * (m0 + 1))

            plan = [("full", i) for i in range(0, nfull, 2)]
            plan += [("diag", 0), ("diag", 1)]
            nplan = len(plan)
            for s in range(nplan + PV_LAG):
                if s < nplan:
                    kind, arg = plan[s]
                    if kind == "full":
                        emit_full_pair(arg)
                    else:
                        emit_diag_pair(arg)
                    pump()
                if s >= PV_LAG:
                    # flipped PV for the two k-chunks of step s - PV_LAG:
                    # per live q-sub, one [128, 65] matmul with lhsT =
                    # the est q-sub slice.  The very first matmul carries
                    # the PSUM start flag (bank-granular lazy zeroing);
                    # everything later lands on zeroed-or-written bytes.
                    sp = s - PV_LAG
                    for ip in range(2 * sp, 2 * sp + 2):
                        a_, sh = ests[ip]
                        for n in range(max(0, ip - 4 * j), 4):
                            nc.tensor.matmul(
                                pso[:, n, 0:65],
                                lhsT=a_[:, 128 * n - sh : 128 * n - sh + 128],
                                rhs=v_sb[:, ip, h, :],
                                start=(ip == 0 and n == 0),
                                stop=(ip == nk - 1 and n == 3),
                                skip_group_check=True,
                            )
            # normalize: denominators sit in column 64 of each q-sub
            # accumulator -- one batched reciprocal, then per-sub
            # per-partition scaling into the [q, ch] staging tile
            if h % 2 == 0:
                stages[pt] = pstg.tile([128, 4, 128], BF16, name="stg", tag="stg")
            stage = stages[pt]
            rec4 = prec.tile([128, 4], F32, name="rec4", tag="rec4")
            nc.vector.reciprocal(
                rec4, pso[:, :, 64:65].rearrange("p n o -> p (n o)")
            )
            for n in range(4):
                nc.vector.tensor_scalar_mul(
                    stage[:, n, off : off + DH],
                    pso[:, n, 0:DH],
                    rec4[:, n : n + 1],
                )
            if h % 2 == 1 and not tail:
                # XBAR DMA-transpose [q, (sub, ch)] -> [ch, (sub, q)]:
                # lands the pair's attn-out in output-projection layout
                # with zero PE/DVE work
                nc.sync.dma_start_transpose(
                    aot_sb[pt][j].rearrange("p (n q) -> p n q", n=4), stage
                )

        def finalize_tail_pair(pt, j):
            """PE-transpose path for the last pair: shorter latency than
            the XBAR DMA-transpose, and PE is idle at this point."""
            stage = stages[pt]
            ptile = psop.tile([128, 512], BF16, name="ptile", tag="pso")
            for n in range(4):
                nc.tensor.matmul(
                    ptile[:, 128 * n : 128 * n + 128],
                    lhsT=stage[:, n, :],
                    rhs=ident_sb,
                    is_transpose=True,
                    start=True,
                    stop=True,
                )
            nc.vector.tensor_copy(aot_sb[pt][j], ptile)

        # j-major schedule: QKV strips and output-projection chunks are
        # threaded through the attention rounds as PE filler, weighted
        # toward the later (bigger, exp-bound) rounds.
        # p-state warmups: keep PE busy (and ramping) while the first
        # xt/wq transfers land; results are never read
        for _ in range(9):
            psw = pmm.tile([128, 512], F32, name="psw", tag="mm")
            nc.tensor.matmul(
                psw, lhsT=wdum_sb[:, 0:128], rhs=wdum_sb, start=True, stop=True
            )
        for _ in qkv_strip_units(0):
            pass
        FILLER_PLAN = {
            0: [1],
            1: [2],
            2: [3],
            3: [-1, -2, -3, "p3a"],
        }
        for j in range(NTS):
            gens = []
            n_units = 0
            for f in FILLER_PLAN[j]:
                if f == "p3a":
                    gens.append(proj_last_part1(NTS - 1))
                    n_units += 8
                elif f >= 0:
                    gens.append(qkv_strip_units(f))
                    n_units += 12
                else:
                    gens.append(proj_units(-f - 1))
                    n_units += 8
            # in the last round, hold back a few filler units to run
            # after the final attention block, covering the last pair's
            # normalize-chain latency before the output projection
            reserve = 2 if j == NTS - 1 else 0
            usable = max(1, n_units - reserve)
            pump_calls = HL * (2 * j + 2)
            quota = (usable / max(1, pump_calls - 3)) if n_units else 0.0

            def _advance():
                while gens:
                    try:
                        next(gens[0])
                        return
                    except StopIteration:
                        gens.pop(0)

            state = {"cnt": 0, "used": 0, "credit": 0.0}

            def pump():
                state["cnt"] += 1
                # let the strip's xt DMAs land before filler matmuls;
                # stop at the usable budget so `reserve` units remain
                # for the post-round drain
                if state["cnt"] < 3:
                    return
                state["credit"] += quota
                while state["credit"] >= 1.0 and state["used"] < usable:
                    state["credit"] -= 1.0
                    state["used"] += 1
                    _advance()

            for h in range(HL):
                attn(h, j, pump, tail=(j == NTS - 1 and h == HL - 1))
            for g in gens:
                for _ in g:
                    pass
            if j == NTS - 1:
                finalize_tail_pair(HL // 2 - 1, j)
        for _ in proj_last_part2(NTS - 1):
            pass


_PROGRAM = None


def _get_program():
    global _PROGRAM
    if _PROGRAM is None:
        _PROGRAM = build_program()
    return _PROGRAM


def _make_packed_masks():
    """Causal 0/1 masks for the packed diagonal-pair est tiles.
    mska[i, c] allows (c >= i) for c < 512 (diag m=0) and (c-512 >= i)
    for c >= 512 (diag m=1); mskb is the same at half scale (m=2, 3)."""
    i = np.arange(128)[:, None]
    ca = np.arange(896)[None, :]
    mska = np.where(ca < 512, ca >= i, (ca - 512) >= i)
    cb = np.arange(384)[None, :]
    mskb = np.where(cb < 256, cb >= i, (cb - 256) >= i)
    bf = ml_dtypes.bfloat16
    return mska.astype(bf), mskb.astype(bf)


def make_in_maps(x, W_qkv, b_qkv, W_proj):
    """Shard the full inputs into the 8 per-core input maps."""
    x = np.asarray(x, dtype=np.float32)
    W_qkv = np.asarray(W_qkv, dtype=np.float32)
    b_qkv = np.asarray(b_qkv, dtype=np.float32)
    W_proj = np.asarray(W_proj, dtype=np.float32)
    bf = ml_dtypes.bfloat16
    mska, mskb = _make_packed_masks()
    ident = np.eye(128, dtype=bf)
    in_maps = []
    for core in range(NCORES):
        b, g = core // 2, core % 2
        cs = slice(CL * g, CL * g + CL)
        xt = np.ascontiguousarray(
            x[b].T.reshape(C, T // 512, 512).transpose(1, 0, 2)
        ).astype(bf)
        wq_s = np.ascontiguousarray(W_qkv[:, CL * g : CL * g + CL]).astype(bf)
        wk_s = np.ascontiguousarray(W_qkv[:, C + CL * g : C + CL * g + CL]).astype(bf)
        wv_s = np.ascontiguousarray(
            W_qkv[:, 2 * C + CL * g : 2 * C + CL * g + CL]
        ).astype(bf)
        wp_s = np.ascontiguousarray(W_proj[CL * g : CL * g + CL, :]).astype(bf)
        bq_s = np.ascontiguousarray(b_qkv[cs].reshape(4, 128).T)
        bk_s = np.ascontiguousarray(b_qkv[C + CL * g : C + CL * g + CL].reshape(4, 128).T)
        in_maps.append(
            {
                "xt": xt,
                "wq": wq_s,
                "wk": wk_s,
                "wv": wv_s,
                "wp": wp_s,
                "bq": bq_s,
                "bk": bk_s,
                "mska": mska,
                "mskb": mskb,
                "ident": ident,
            }
        )
    return in_maps


def gather_output(results, b_qkv, W_proj, b_proj):
    """Sum the per-core partial outputs and fold in the host-side biases."""
    b_qkv = np.asarray(b_qkv, dtype=np.float32)
    W_proj = np.asarray(W_proj, dtype=np.float32)
    b_proj = np.asarray(b_proj, dtype=np.float32)
    bv = b_qkv[2 * C : 3 * C]
    extra = (bv @ W_proj + b_proj).astype(np.float32)
    out = np.empty((B, T, C), dtype=np.float32)
    for b in range(B):
        out[b] = (
            np.asarray(results[2 * b]["out"], dtype=np.float32)
            + np.asarray(results[2 * b + 1]["out"], dtype=np.float32)
            + extra
        )
    return out


def kernel(x, W_qkv, b_qkv, W_proj, b_proj):
    nc = _get_program()
    in_maps = make_in_maps(x, W_qkv, b_qkv, W_proj)
    res = run_bass_kernel_spmd(nc, in_maps, list(range(NCORES)))
    return gather_output(res.results, b_qkv, W_proj, b_proj)


# revision 19
# speedup vs baseline: 1.0035x; 1.0035x over previous
"""TRN2 Bass kernel: causal multi-head self-attention block (QKV proj ->
causal softmax attention -> output proj) for B=4, T=2048, C=1024, H=16.

Sharding over 8 NeuronCores: core c handles batch b = c//2 and head-group
g = c%2 (8 of the 16 heads, i.e. 512 of the 1024 hidden channels).  Each
core computes its batch's QKV projection restricted to its head-group's
columns, causal attention for its 8 heads, and a *partial* output
projection (its 512 rows of W_proj).  The host sums the two partial
outputs per batch and adds the host-foldable bias terms
(b_proj, and b_qkv's V part folded through W_proj).

Per-core device kernel (bf16 matmuls, fp32 PSUM accumulation):
  - x arrives pre-transposed (xT, [C,T]) so the QKV projection needs no
    on-device transpose; Q^T/K^T are produced in [d_head, T] layout,
    which is exactly the scores-matmul operand layout (contraction over
    d on the partition axis): S^T blocks [128 k, 512 q] are one matmul
    each, with causal block skipping and diagonal blocks shortened to
    their live [delta:512] q-range.
  - exp() runs on the Scalar engine straight out of PSUM (the 1/sqrt(64)
    scale folded in); adjacent full blocks share a 2-bank PSUM tile and
    a single exp; diagonal blocks are causal-masked after the exp by a
    0/1 bf16 strip multiply (2x DVE mode).
  - PV runs *flipped*: out[q,d] tiles [128, 65] with lhsT = est (the
    exp'd scores block, q-sub-sliced) and rhs = V (with a trailing
    all-ones column per head so the softmax denominator accumulates in
    column 64 for free).  A flipped-PV matmul costs 65 PE rows instead
    of the 512 an [65, 512] block costs, and the denominator lands
    per-partition, so normalization is one batched reciprocal plus
    native tensor_scalar multiplies -- no partition broadcast needed.
  - The normalized attn-out staging tiles [q, ch] are moved into the
    [ch, q] layout the output projection needs by a single XBAR
    DMA-transpose per head-pair and strip (SBUF->SBUF, no PE/DVE work);
    the very last pair instead uses PE transposes + a DVE copy to keep
    the tail off the long DMA-transpose latency.
  - Scheduling: the score/exp stream is software-pipelined ahead of the
    PV matmuls, and QKV-strip / output-projection work is threaded
    through the attention rounds as PE filler (weighted toward the
    later, exp-heavy rounds) so the Tensor engine stays busy while the
    Scalar engine works through the exp stream.
  - DMA: weights/activations load as one descriptor-batched DMA each
    (HWDGE issue overhead dominates many-small-DMA plans); the strip-0
    x and W_q transfers are split in halves so the first projection
    matmuls can start as early as possible.

Toolchain workarounds (this container's walrus build): instructions may
carry at most one sync wait, so excess waits are moved onto InstNoOp
wait-carriers (_split_sync_waits), including on the TileContext tail
drain (_SplitDrainTileContext).
"""

import numpy as np
import ml_dtypes

import concourse.bass as bass
import concourse.mybir as mybir
import concourse.tile as tile
from concourse.bass_utils import run_bass_kernel_spmd
from concourse.vector_clock import ScopedClock

# ---------------------------------------------------------------- problem dims
B = 4
T = 2048
C = 1024
H = 16
DH = 64           # head dim
NCORES = 8
HL = H // 2       # heads per core (head-group of 8)
CL = HL * DH      # 512 local channels per core

F32 = mybir.dt.float32
BF16 = mybir.dt.bfloat16
AF = mybir.ActivationFunctionType
SCALE = 1.0 / np.sqrt(DH)

PVW = 68          # per-q-sub accumulator stride in the pso tile (65 used)


class _SplitDrainTileContext(tile.TileContext):
    """TileContext whose tail drain splits its semaphore waits across
    multiple drain instructions; the walrus build in this container
    rejects CTRL instructions carrying more than ~2 sync waits."""

    MAX_WAITS = 1

    def _drain_and_barrier(self, tick_clock, wait_clock):
        nc = self.nc
        drain_inst = nc.sync.drain()
        wait_clock.add_sem_waits(
            drain_inst.ins, ScopedClock({None: tick_clock.global_clock})
        )
        si = drain_inst.ins.sync_info
        waits = list(si.on_wait or []) if si else []
        if len(waits) > self.MAX_WAITS:
            drain_inst.ins.sync_info = mybir.SyncInfo(
                on_wait=waits[: self.MAX_WAITS],
                on_update=list(si.on_update or []),
            )
            rest = waits[self.MAX_WAITS:]
            for i in range(0, len(rest), self.MAX_WAITS):
                extra = nc.sync.drain()
                extra.ins.sync_info = mybir.SyncInfo(
                    on_wait=rest[i : i + self.MAX_WAITS], on_update=[]
                )
        nc.all_engine_barrier()
        assert self.sems is not None
        popped = nc._tile_sem_poison_stack.pop()
        assert popped is self._sem_poison
        nc.clear_and_free_semaphores(list(self.sems.allocated().values()))
        nc.all_engine_barrier()


def _persist(pp, shape, dtype, name):
    return pp.tile(shape, dtype, name=name, tag=name)


_MAX_WAITS = 1


def _split_sync_waits(nc):
    """The walrus build here accepts only a small number of sync waits per
    instruction.  Move excess waits onto InstNoOp wait-carriers inserted
    just before the over-subscribed instruction on the same engine."""
    for bb in nc.main_func.blocks:
        new_insts = []
        changed = False
        for ins in bb.instructions:
            si = ins.sync_info
            waits = list(si.on_wait or []) if si else []
            if len(waits) > _MAX_WAITS:
                changed = True
                extra, keep = waits[_MAX_WAITS:], waits[: _MAX_WAITS]
                for i in range(0, len(extra), _MAX_WAITS):
                    nop = mybir.InstNoOp(name=f"I-waitsplit-{nc.next_id()}")
                    nop.engine = ins.engine
                    nop.sync_info = mybir.SyncInfo(
                        on_wait=extra[i : i + _MAX_WAITS], on_update=[]
                    )
                    new_insts.append(nop)
                ins.sync_info = mybir.SyncInfo(
                    on_wait=keep, on_update=list(si.on_update or [])
                )
            new_insts.append(ins)
        if changed:
            bb.instructions = new_insts
    return nc


def build_program(loop_reps: int = 1):
    """Build the per-core program.  loop_reps > 1 unrolls the whole body
    that many times in one NEFF (used only for wall-clock timing; the
    grading path uses 1)."""
    nc = bass.Bass(trn_type="TRN2")

    xt = nc.dram_tensor("xt", [T // 512, C, 512], BF16, kind="ExternalInput")
    wq = nc.dram_tensor("wq", [C, CL], BF16, kind="ExternalInput")
    wk = nc.dram_tensor("wk", [C, CL], BF16, kind="ExternalInput")
    wv = nc.dram_tensor("wv", [C, CL], BF16, kind="ExternalInput")
    wp = nc.dram_tensor("wp", [CL, C], BF16, kind="ExternalInput")
    bq = nc.dram_tensor("bq", [128, 4], F32, kind="ExternalInput")
    bk = nc.dram_tensor("bk", [128, 4], F32, kind="ExternalInput")
    mska = nc.dram_tensor("mska", [128, 896], BF16, kind="ExternalInput")
    mskb = nc.dram_tensor("mskb", [128, 384], BF16, kind="ExternalInput")
    ident = nc.dram_tensor("ident", [128, 128], BF16, kind="ExternalInput")
    out = nc.dram_tensor("out", [T, C], BF16, kind="ExternalOutput")

    NCC = C // 128            # 8 c-chunks of the model dim
    NTC = T // 128            # 16 t-chunks

    with _SplitDrainTileContext(nc) as tc, tc.tile_pool(
        name="persist", bufs=1
    ) as pp:
        # ------------------------------------------------ persistent SBUF
        xt_sb = _persist(pp, [128, NCC, T], BF16, "xts")
        wq_sb = _persist(pp, [128, NCC, CL], BF16, "wqs")
        wk_sb = _persist(pp, [128, NCC, CL], BF16, "wks")
        wv_sb = _persist(pp, [128, NCC, CL], BF16, "wvs")
        wp_sb = _persist(pp, [128, CL // 128, C], BF16, "wps")
        bq_sb = _persist(pp, [128, 4], F32, "bqs")
        bk_sb = _persist(pp, [128, 4], F32, "bks")
        mska_sb = _persist(pp, [128, 896], BF16, "mskas")
        mskb_sb = _persist(pp, [128, 384], BF16, "mskbs")
        ident_sb = _persist(pp, [128, 128], BF16, "idents")
        wdum_sb = _persist(pp, [128, 512], BF16, "wdums")
        qt_sb = [_persist(pp, [128, T], BF16, f"qts{p}") for p in range(4)]
        kt_sb = [_persist(pp, [128, T], BF16, f"kts{p}") for p in range(4)]
        # V with a trailing ones column per head: [t-part, t-chunk, head, 64+1]
        v_sb = _persist(pp, [128, NTC, HL, DH + 1], BF16, "vsb")
        aot_sb = [
            [_persist(pp, [128, 512], BF16, f"aots{p}_{jj}") for jj in range(T // 512)]
            for p in range(4)
        ]

        for _rep in range(loop_reps):
            _emit_body(
                nc, tc, xt, wq, wk, wv, wp, bq, bk, mska, mskb, ident, out,
                xt_sb, wq_sb, wk_sb, wv_sb, wp_sb, bq_sb, bk_sb, mska_sb,
                mskb_sb, ident_sb, wdum_sb, qt_sb, kt_sb, v_sb, aot_sb,
            )
    _split_sync_waits(nc)
    return nc


def _emit_body(
    nc, tc, xt, wq, wk, wv, wp, bq, bk, mska, mskb, ident, out,
    xt_sb, wq_sb, wk_sb, wv_sb, wp_sb, bq_sb, bk_sb, mska_sb,
    mskb_sb, ident_sb, wdum_sb, qt_sb, kt_sb, v_sb, aot_sb,
):
    NCC = C // 128
    NTS = T // 512

    def _rows(w, lo, hi):
        # w[lo:hi, :] laid out [p, chunk, outcol]
        return w[lo:hi].rearrange("(c p) o -> p c o", p=128)

    # All large transfers on the sync HWDGE queue in exact need-order, so
    # the (serial) DMA-engine track processes them in the order the PE
    # stream consumes them.  Tiny transfers ride the scalar queue.
    nc.sync.dma_start(
        out=xt_sb[:, 0:4, 0:512], in_=xt[0, 0:512].rearrange("(c p) t -> p c t", p=128)
    )
    nc.sync.dma_start(
        out=wq_sb[:, :, 0:256],
        in_=wq[:, 0:256].rearrange("(c p) o -> p c o", p=128),
    )
    nc.sync.dma_start(
        out=xt_sb[:, 4:8, 0:512],
        in_=xt[0, 512:1024].rearrange("(c p) t -> p c t", p=128),
    )
    nc.sync.dma_start(
        out=wq_sb[:, :, 256:512],
        in_=wq[:, 256:512].rearrange("(c p) o -> p c o", p=128),
    )
    nc.sync.dma_start(out=wk_sb, in_=_rows(wk, 0, C))
    nc.sync.dma_start(out=wv_sb, in_=_rows(wv, 0, C))
    for j in range(1, NTS):
        nc.sync.dma_start(
            out=xt_sb[:, :, 512 * j : 512 * j + 512],
            in_=xt[j].rearrange("(c p) t -> p c t", p=128),
        )
    nc.sync.dma_start(out=wp_sb, in_=wp.rearrange("(c p) o -> p c o", p=128))
    nc.scalar.dma_start(out=bq_sb, in_=bq[:])
    nc.scalar.dma_start(out=bk_sb, in_=bk[:])
    nc.scalar.dma_start(out=mska_sb, in_=mska[:])
    nc.scalar.dma_start(out=mskb_sb, in_=mskb[:])
    nc.scalar.dma_start(out=ident_sb, in_=ident[:])
    nc.vector.memset(wdum_sb, 0.0)
    nc.vector.memset(v_sb[:, :, :, DH : DH + 1], 1.0)

    # ------------------------------------------------ pools
    with (
        tc.tile_pool(name="pmm", bufs=2, space="PSUM") as pmm,
        tc.tile_pool(name="pst", bufs=2, space="PSUM") as pstp,
        tc.tile_pool(name="pso", bufs=2, space="PSUM") as psop,
        tc.tile_pool(name="pest", bufs=10) as pest,
        tc.tile_pool(name="pstg", bufs=3) as pstg,
        tc.tile_pool(name="prec", bufs=4) as prec,
        tc.tile_pool(name="pout", bufs=3) as pout,
    ):

        def qk_strip_units(j):
            """Q^T/K^T projection for t-strip j, yielded in PE-sized
            units so attention rounds can interleave them as filler.
            Strip 0 runs all Q before all K so the first matmuls only
            depend on the wq/xt transfers."""
            t0 = 512 * j
            plan = (
                [(p, 0) for p in range(4)] + [(p, 1) for p in range(4)]
                if j == 0
                else [(p, qk) for p in range(4) for qk in range(2)]
            )
            for p, qk in plan:
                w_sb, b_sb, o_sb = (
                    (wq_sb, bq_sb, qt_sb) if qk == 0 else (wk_sb, bk_sb, kt_sb)
                )
                ps = pmm.tile([128, 512], F32, name="psqk", tag="mm")
                for cc in range(NCC):
                    nc.tensor.matmul(
                        ps,
                        lhsT=w_sb[:, cc, 128 * p : 128 * p + 128],
                        rhs=xt_sb[:, cc, t0 : t0 + 512],
                        start=(cc == 0),
                        stop=(cc == NCC - 1),
                    )
                nc.vector.tensor_scalar_add(
                    o_sb[p][:, t0 : t0 + 512], ps, b_sb[:, p : p + 1]
                )
                yield

        def v_strip_units(j):
            """V projection for the 4 t-chunks of strip j."""
            for ic in range(4 * j, 4 * j + 4):
                psv = pmm.tile([128, 512], F32, name="psv", tag="mm")
                for cc in range(NCC):
                    nc.tensor.matmul(
                        psv,
                        lhsT=xt_sb[:, cc, 128 * ic : 128 * ic + 128],
                        rhs=wv_sb[:, cc, :],
                        start=(cc == 0),
                        stop=(cc == NCC - 1),
                    )
                nc.vector.tensor_copy(
                    v_sb[:, ic, :, 0:DH],
                    psv.rearrange("p (h d) -> p h d", h=HL),
                )
                yield

        def qkv_strip_units(j):
            yield from qk_strip_units(j)
            yield from v_strip_units(j)

        def _proj_mm(j, qi, nh, ccs):
            ps3 = pmm.tile([128, 512], F32, name="ps3", tag="mm")
            for ci, cc in enumerate(ccs):
                nc.tensor.matmul(
                    ps3,
                    lhsT=aot_sb[cc][j][:, 128 * (qi % 4) : 128 * (qi % 4) + 128],
                    rhs=wp_sb[:, cc, 512 * nh : 512 * nh + 512],
                    start=(ci == 0),
                    stop=(ci == len(ccs) - 1),
                )
            return ps3

        def proj_units(j):
            """Partial output projection for the 4 t-chunks of strip j."""
            for qi in range(4 * j, 4 * j + 4):
                ostg = pout.tile([128, C], BF16, name="ostg", tag="ostg")
                for nh in range(2):
                    ps3 = _proj_mm(j, qi, nh, range(CL // 128))
                    nc.vector.tensor_copy(ostg[:, 512 * nh : 512 * nh + 512], ps3)
                    nc.sync.dma_start(
                        out=out[128 * qi : 128 * qi + 128, 512 * nh : 512 * nh + 512],
                        in_=ostg[:, 512 * nh : 512 * nh + 512],
                    )
                    yield

        lstg = {}  # (qi, nh) -> bf16 staging of the cc{0,1} half-sum

        def proj_last_part1(j):
            """First half-accumulation (pairs 0,1) of the last strip's
            projection -- runs as round-3 filler once aot[0..1][j] exist,
            adding PE work to the exp-bound closing round."""
            for qi in range(4 * j, 4 * j + 4):
                for nh in range(2):
                    ps3 = _proj_mm(j, qi, nh, (0, 1))
                    st_ = pout.tile([128, 512], BF16, name="lstg", tag="lstg", bufs=16)
                    nc.vector.tensor_copy(st_, ps3)
                    lstg[(qi, nh)] = st_
                    yield

        def proj_last_part2(j):
            """Second half (pairs 2,3) + combine + store."""
            for qi in range(4 * j, 4 * j + 4):
                ostg = pout.tile([128, C], BF16, name="ostg", tag="ostg")
                for nh in range(2):
                    ps3 = _proj_mm(j, qi, nh, (2, 3))
                    nc.vector.tensor_add(
                        ostg[:, 512 * nh : 512 * nh + 512], ps3, lstg[(qi, nh)]
                    )
                    nc.sync.dma_start(
                        out=out[128 * qi : 128 * qi + 128, 512 * nh : 512 * nh + 512],
                        in_=ostg[:, 512 * nh : 512 * nh + 512],
                    )
                    yield

        PV_LAG = 4  # plan steps (2 k-chunks each) the PV trails behind

        stages = {}  # pair -> stage tile for the pair currently in flight

        def attn(h, j, pump, tail=False):
            """Causal attention for head h over q-strip j.  The score/exp
            stream is software-pipelined PV_LAG plan-steps (2 k-chunks
            each) ahead of the flipped PV accumulation; the four
            diagonal blocks are packed into two PSUM tiles so they cost
            two exp/mask instructions instead of four; `pump` is called
            once per step to emit filler projection work that keeps PE
            busy while the Scalar engine works through the exp stream."""
            pt, off = h // 2, (h % 2) * DH
            qt_h = qt_sb[pt][off : off + DH, :]
            kt_h = kt_sb[pt][off : off + DH, :]
            q0 = 512 * j
            nk = 4 * (j + 1)
            pso = psop.tile([128, 4, PVW], F32, name="pso", tag="pso")
            # ests[ip] = (ap, shift): PV lhsT for q-sub n is
            # ap[:, 128n - shift : 128n - shift + 128]
            ests = [None] * nk
            nfull = 4 * j

            def emit_full_pair(i):
                pst2 = pstp.tile([128, 1024], F32, name="pst2", tag="st")
                for half in range(2):
                    nc.tensor.matmul(
                        pst2[:, 512 * half : 512 * half + 512],
                        lhsT=kt_h[:, 128 * (i + half) : 128 * (i + half) + 128],
                        rhs=qt_h[:, q0 : q0 + 512],
                        start=True,
                        stop=True,
                    )
                est2 = pest.tile([128, 1024], BF16, name="est2", tag="est")
                nc.scalar.activation(est2, pst2, AF.Exp, scale=SCALE)
                ests[i] = (est2[:, 0:512], 0)
                ests[i + 1] = (est2[:, 512:1024], 0)

            def emit_diag_pair(pos):
                # pos 0: diagonals m=0,1 packed at cols [0:512 | 512:896]
                # pos 1: diagonals m=2,3 packed at cols [0:256 | 256:384]
                m0 = 2 * pos
                w0 = 512 - 128 * m0          # live width of diag m0
                w1 = w0 - 128                # live width of diag m0+1
                pst2 = pstp.tile([128, 1024], F32, name="pst2", tag="st")
                nc.tensor.matmul(
                    pst2[:, 0:w0],
                    lhsT=kt_h[:, 128 * (nfull + m0) : 128 * (nfull + m0) + 128],
                    rhs=qt_h[:, q0 + 128 * m0 : q0 + 512],
                    start=True,
                    stop=True,
                )
                nc.tensor.matmul(
                    pst2[:, w0 : w0 + w1],
                    lhsT=kt_h[:, 128 * (nfull + m0 + 1) : 128 * (nfull + m0 + 1) + 128],
                    rhs=qt_h[:, q0 + 128 * (m0 + 1) : q0 + 512],
                    start=True,
                    stop=True,
                )
                est2 = pest.tile([128, 1024], BF16, name="est2", tag="est")
                nc.scalar.activation(
                    est2[:, 0 : w0 + w1], pst2[:, 0 : w0 + w1], AF.Exp, scale=SCALE
                )
                mk = mska_sb if pos == 0 else mskb_sb
                nc.vector.tensor_mul(
                    est2[:, 0 : w0 + w1], est2[:, 0 : w0 + w1], mk[:, 0 : w0 + w1]
                )
                ests[nfull + m0] = (est2[:, 0:w0], 128 * m0)
                ests[nfull + m0 + 1] = (est2[:, w0 : w0 + w1], 128 * (m0 + 1))

            plan = [("full", i) for i in range(0, nfull, 2)]
            plan += [("diag", 0), ("diag", 1)]
            nplan = len(plan)
            for s in range(nplan + PV_LAG):
                if s < nplan:
                    kind, arg = plan[s]
                    if kind == "full":
                        emit_full_pair(arg)
                    else:
                        emit_diag_pair(arg)
                    pump()
                if s >= PV_LAG:
                    # flipped PV for the two k-chunks of step s - PV_LAG:
                    # per live q-sub, one [128, 65] matmul with lhsT =
                    # the est q-sub slice.  The very first matmul carries
                    # the PSUM start flag (bank-granular lazy zeroing);
                    # everything later lands on zeroed-or-written bytes.
                    sp = s - PV_LAG
                    for ip in range(2 * sp, 2 * sp + 2):
                        a_, sh = ests[ip]
                        for n in range(max(0, ip - 4 * j), 4):
                            nc.tensor.matmul(
                                pso[:, n, 0:65],
                                lhsT=a_[:, 128 * n - sh : 128 * n - sh + 128],
                                rhs=v_sb[:, ip, h, :],
                                start=(ip == 0 and n == 0),
                                stop=(ip == nk - 1 and n == 3),
                                skip_group_check=True,
                            )
            # normalize: denominators sit in column 64 of each q-sub
            # accumulator -- one batched reciprocal, then per-sub
            # per-partition scaling into the [q, ch] staging tile
            if h % 2 == 0:
                stages[pt] = pstg.tile([128, 4, 128], BF16, name="stg", tag="stg")
            stage = stages[pt]
            rec4 = prec.tile([128, 4], F32, name="rec4", tag="rec4")
            nc.vector.reciprocal(
                rec4, pso[:, :, 64:65].rearrange("p n o -> p (n o)")
            )
            for n in range(4):
                nc.vector.tensor_scalar_mul(
                    stage[:, n, off : off + DH],
                    pso[:, n, 0:DH],
                    rec4[:, n : n + 1],
                )
            if h % 2 == 1 and not tail:
                # XBAR DMA-transpose [q, (sub, ch)] -> [ch, (sub, q)]:
                # lands the pair's attn-out in output-projection layout
                # with zero PE/DVE work
                nc.sync.dma_start_transpose(
                    aot_sb[pt][j].rearrange("p (n q) -> p n q", n=4), stage
                )

        def finalize_tail_pair(pt, j):
            """PE-transpose path for the last pair: shorter latency than
            the XBAR DMA-transpose, and PE is idle at this point."""
            stage = stages[pt]
            ptile = psop.tile([128, 512], BF16, name="ptile", tag="pso")
            for n in range(4):
                nc.tensor.matmul(
                    ptile[:, 128 * n : 128 * n + 128],
                    lhsT=stage[:, n, :],
                    rhs=ident_sb,
                    is_transpose=True,
                    start=True,
                    stop=True,
                )
            nc.vector.tensor_copy(aot_sb[pt][j], ptile)

        # j-major schedule: QKV strips and output-projection chunks are
        # threaded through the attention rounds as PE filler, weighted
        # toward the later (bigger, exp-bound) rounds.
        # p-state warmups: keep PE busy (and ramping) while the first
        # xt/wq transfers land; results are never read
        for _ in range(9):
            psw = pmm.tile([128, 512], F32, name="psw", tag="mm")
            nc.tensor.matmul(
                psw, lhsT=wdum_sb[:, 0:128], rhs=wdum_sb, start=True, stop=True
            )
        for _ in qkv_strip_units(0):
            pass
        FILLER_PLAN = {
            0: [1],
            1: [2],
            2: [3],
            3: [-1, -2, -3, "p3a"],
        }
        for j in range(NTS):
            gens = []
            n_units = 0
            for f in FILLER_PLAN[j]:
                if f == "p3a":
                    gens.append(proj_last_part1(NTS - 1))
                    n_units += 8
                elif f >= 0:
                    gens.append(qkv_strip_units(f))
                    n_units += 12
                else:
                    gens.append(proj_units(-f - 1))
                    n_units += 8
            # in the last round, hold back a few filler units to run
            # after the final attention block, covering the last pair's
            # normalize-chain latency before the output projection
            reserve = 2 if j == NTS - 1 else 0
            usable = max(1, n_units - reserve)
            pump_calls = HL * (2 * j + 2)
            quota = (usable / max(1, pump_calls - 3)) if n_units else 0.0

            def _advance():
                while gens:
                    try:
                        next(gens[0])
                        return
                    except StopIteration:
                        gens.pop(0)

            state = {"cnt": 0, "used": 0, "credit": 0.0}

            def pump():
                state["cnt"] += 1
                # let the strip's xt DMAs land before filler matmuls;
                # stop at the usable budget so `reserve` units remain
                # for the post-round drain
                if state["cnt"] < 3:
                    return
                state["credit"] += quota
                while state["credit"] >= 1.0 and state["used"] < usable:
                    state["credit"] -= 1.0
                    state["used"] += 1
                    _advance()

            for h in range(HL):
                attn(h, j, pump, tail=(j == NTS - 1 and h == HL - 1))
            for g in gens:
                for _ in g:
                    pass
            if j == NTS - 1:
                finalize_tail_pair(HL // 2 - 1, j)
        for _ in proj_last_part2(NTS - 1):
            pass


_PROGRAM = None


def _get_program():
    global _PROGRAM
    if _PROGRAM is None:
        _PROGRAM = build_program()
    return _PROGRAM


def _make_packed_masks():
    """Causal 0/1 masks for the packed diagonal-pair est tiles.
    mska[i, c] allows (c >= i) for c < 512 (diag m=0) and (c-512 >= i)
    for c >= 512 (diag m=1); mskb is the same at half scale (m=2, 3)."""
    i = np.arange(128)[:, None]
    ca = np.arange(896)[None, :]
    mska = np.where(ca < 512, ca >= i, (ca - 512) >= i)
    cb = np.arange(384)[None, :]
    mskb = np.where(cb < 256, cb >= i, (cb - 256) >= i)
    bf = ml_dtypes.bfloat16
    return mska.astype(bf), mskb.astype(bf)


def make_in_maps(x, W_qkv, b_qkv, W_proj):
    """Shard the full inputs into the 8 per-core input maps."""
    x = np.asarray(x, dtype=np.float32)
    W_qkv = np.asarray(W_qkv, dtype=np.float32)
    b_qkv = np.asarray(b_qkv, dtype=np.float32)
    W_proj = np.asarray(W_proj, dtype=np.float32)
    bf = ml_dtypes.bfloat16
    mska, mskb = _make_packed_masks()
    ident = np.eye(128, dtype=bf)
    in_maps = []
    for core in range(NCORES):
        b, g = core // 2, core % 2
        cs = slice(CL * g, CL * g + CL)
        xt = np.ascontiguousarray(
            x[b].T.reshape(C, T // 512, 512).transpose(1, 0, 2)
        ).astype(bf)
        wq_s = np.ascontiguousarray(W_qkv[:, CL * g : CL * g + CL]).astype(bf)
        wk_s = np.ascontiguousarray(W_qkv[:, C + CL * g : C + CL * g + CL]).astype(bf)
        wv_s = np.ascontiguousarray(
            W_qkv[:, 2 * C + CL * g : 2 * C + CL * g + CL]
        ).astype(bf)
        wp_s = np.ascontiguousarray(W_proj[CL * g : CL * g + CL, :]).astype(bf)
        bq_s = np.ascontiguousarray(b_qkv[cs].reshape(4, 128).T)
        bk_s = np.ascontiguousarray(b_qkv[C + CL * g : C + CL * g + CL].reshape(4, 128).T)
        in_maps.append(
            {
                "xt": xt,
                "wq": wq_s,
                "wk": wk_s,
                "wv": wv_s,
                "wp": wp_s,
                "bq": bq_s,
                "bk": bk_s,
                "mska": mska,
                "mskb": mskb,
                "ident": ident,
            }
        )
    return in_maps


def gather_output(results, b_qkv, W_proj, b_proj):
    """Sum the per-core partial outputs and fold in the host-side biases."""
    b_qkv = np.asarray(b_qkv, dtype=np.float32)
    W_proj = np.asarray(W_proj, dtype=np.float32)
    b_proj = np.asarray(b_proj, dtype=np.float32)
    bv = b_qkv[2 * C : 3 * C]
    extra = (bv @ W_proj + b_proj).astype(np.float32)
    out = np.empty((B, T, C), dtype=np.float32)
    for b in range(B):
        out[b] = (
            np.asarray(results[2 * b]["out"], dtype=np.float32)
            + np.asarray(results[2 * b + 1]["out"], dtype=np.float32)
            + extra
        )
    return out


def kernel(x, W_qkv, b_qkv, W_proj, b_proj):
    nc = _get_program()
    in_maps = make_in_maps(x, W_qkv, b_qkv, W_proj)
    res = run_bass_kernel_spmd(nc, in_maps, list(range(NCORES)))
    return gather_output(res.results, b_qkv, W_proj, b_proj)


# revision 20
# speedup vs baseline: 1.0037x; 1.0003x over previous
"""TRN2 Bass kernel: causal multi-head self-attention block (QKV proj ->
causal softmax attention -> output proj) for B=4, T=2048, C=1024, H=16.

Sharding over 8 NeuronCores: core c handles batch b = c//2 and head-group
g = c%2 (8 of the 16 heads, i.e. 512 of the 1024 hidden channels).  Each
core computes its batch's QKV projection restricted to its head-group's
columns, causal attention for its 8 heads, and a *partial* output
projection (its 512 rows of W_proj).  The host sums the two partial
outputs per batch and adds the host-foldable bias terms
(b_proj, and b_qkv's V part folded through W_proj).

Per-core device kernel (bf16 matmuls, fp32 PSUM accumulation):
  - x arrives pre-transposed (xT, [C,T]) so the QKV projection needs no
    on-device transpose; Q^T/K^T are produced in [d_head, T] layout,
    which is exactly the scores-matmul operand layout (contraction over
    d on the partition axis): S^T blocks [128 k, 512 q] are one matmul
    each, with causal block skipping and diagonal blocks shortened to
    their live [delta:512] q-range.
  - exp() runs on the Scalar engine straight out of PSUM (the 1/sqrt(64)
    scale folded in); adjacent full blocks share a 2-bank PSUM tile and
    a single exp; diagonal blocks are causal-masked after the exp by a
    0/1 bf16 strip multiply (2x DVE mode).
  - PV runs *flipped*: out[q,d] tiles [128, 65] with lhsT = est (the
    exp'd scores block, q-sub-sliced) and rhs = V (with a trailing
    all-ones column per head so the softmax denominator accumulates in
    column 64 for free).  A flipped-PV matmul costs 65 PE rows instead
    of the 512 an [65, 512] block costs, and the denominator lands
    per-partition, so normalization is one batched reciprocal plus
    native tensor_scalar multiplies -- no partition broadcast needed.
  - The normalized attn-out staging tiles [q, ch] are moved into the
    [ch, q] layout the output projection needs by a single XBAR
    DMA-transpose per head-pair and strip (SBUF->SBUF, no PE/DVE work);
    the very last pair instead uses PE transposes + a DVE copy to keep
    the tail off the long DMA-transpose latency.
  - Scheduling: the score/exp stream is software-pipelined ahead of the
    PV matmuls, and QKV-strip / output-projection work is threaded
    through the attention rounds as PE filler (weighted toward the
    later, exp-heavy rounds) so the Tensor engine stays busy while the
    Scalar engine works through the exp stream.
  - DMA: weights/activations load as one descriptor-batched DMA each
    (HWDGE issue overhead dominates many-small-DMA plans); the strip-0
    x and W_q transfers are split in halves so the first projection
    matmuls can start as early as possible.

Toolchain workarounds (this container's walrus build): instructions may
carry at most one sync wait, so excess waits are moved onto InstNoOp
wait-carriers (_split_sync_waits), including on the TileContext tail
drain (_SplitDrainTileContext).
"""

import numpy as np
import ml_dtypes

import concourse.bass as bass
import concourse.mybir as mybir
import concourse.tile as tile
from concourse.bass_utils import run_bass_kernel_spmd
from concourse.vector_clock import ScopedClock

# ---------------------------------------------------------------- problem dims
B = 4
T = 2048
C = 1024
H = 16
DH = 64           # head dim
NCORES = 8
HL = H // 2       # heads per core (head-group of 8)
CL = HL * DH      # 512 local channels per core

F32 = mybir.dt.float32
BF16 = mybir.dt.bfloat16
AF = mybir.ActivationFunctionType
SCALE = 1.0 / np.sqrt(DH)

PVW = 68          # per-q-sub accumulator stride in the pso tile (65 used)


class _SplitDrainTileContext(tile.TileContext):
    """TileContext whose tail drain splits its semaphore waits across
    multiple drain instructions; the walrus build in this container
    rejects CTRL instructions carrying more than ~2 sync waits."""

    MAX_WAITS = 1

    def _drain_and_barrier(self, tick_clock, wait_clock):
        nc = self.nc
        drain_inst = nc.sync.drain()
        wait_clock.add_sem_waits(
            drain_inst.ins, ScopedClock({None: tick_clock.global_clock})
        )
        si = drain_inst.ins.sync_info
        waits = list(si.on_wait or []) if si else []
        if len(waits) > self.MAX_WAITS:
            drain_inst.ins.sync_info = mybir.SyncInfo(
                on_wait=waits[: self.MAX_WAITS],
                on_update=list(si.on_update or []),
            )
            rest = waits[self.MAX_WAITS:]
            for i in range(0, len(rest), self.MAX_WAITS):
                extra = nc.sync.drain()
                extra.ins.sync_info = mybir.SyncInfo(
                    on_wait=rest[i : i + self.MAX_WAITS], on_update=[]
                )
        nc.all_engine_barrier()
        assert self.sems is not None
        popped = nc._tile_sem_poison_stack.pop()
        assert popped is self._sem_poison
        nc.clear_and_free_semaphores(list(self.sems.allocated().values()))
        nc.all_engine_barrier()


def _persist(pp, shape, dtype, name):
    return pp.tile(shape, dtype, name=name, tag=name)


_MAX_WAITS = 1


def _split_sync_waits(nc):
    """The walrus build here accepts only a small number of sync waits per
    instruction.  Move excess waits onto InstNoOp wait-carriers inserted
    just before the over-subscribed instruction on the same engine."""
    for bb in nc.main_func.blocks:
        new_insts = []
        changed = False
        for ins in bb.instructions:
            si = ins.sync_info
            waits = list(si.on_wait or []) if si else []
            if len(waits) > _MAX_WAITS:
                changed = True
                extra, keep = waits[_MAX_WAITS:], waits[: _MAX_WAITS]
                for i in range(0, len(extra), _MAX_WAITS):
                    nop = mybir.InstNoOp(name=f"I-waitsplit-{nc.next_id()}")
                    nop.engine = ins.engine
                    nop.sync_info = mybir.SyncInfo(
                        on_wait=extra[i : i + _MAX_WAITS], on_update=[]
                    )
                    new_insts.append(nop)
                ins.sync_info = mybir.SyncInfo(
                    on_wait=keep, on_update=list(si.on_update or [])
                )
            new_insts.append(ins)
        if changed:
            bb.instructions = new_insts
    return nc


def build_program(loop_reps: int = 1):
    """Build the per-core program.  loop_reps > 1 unrolls the whole body
    that many times in one NEFF (used only for wall-clock timing; the
    grading path uses 1)."""
    nc = bass.Bass(trn_type="TRN2")

    xt = nc.dram_tensor("xt", [T // 512, C, 512], BF16, kind="ExternalInput")
    wq = nc.dram_tensor("wq", [C, CL], BF16, kind="ExternalInput")
    wk = nc.dram_tensor("wk", [C, CL], BF16, kind="ExternalInput")
    wv = nc.dram_tensor("wv", [C, CL], BF16, kind="ExternalInput")
    wp = nc.dram_tensor("wp", [CL, C], BF16, kind="ExternalInput")
    bq = nc.dram_tensor("bq", [128, 4], F32, kind="ExternalInput")
    bk = nc.dram_tensor("bk", [128, 4], F32, kind="ExternalInput")
    mska = nc.dram_tensor("mska", [128, 896], BF16, kind="ExternalInput")
    mskb = nc.dram_tensor("mskb", [128, 384], BF16, kind="ExternalInput")
    ident = nc.dram_tensor("ident", [128, 128], BF16, kind="ExternalInput")
    out = nc.dram_tensor("out", [T, C], BF16, kind="ExternalOutput")

    NCC = C // 128            # 8 c-chunks of the model dim
    NTC = T // 128            # 16 t-chunks

    with _SplitDrainTileContext(nc) as tc, tc.tile_pool(
        name="persist", bufs=1
    ) as pp:
        # ------------------------------------------------ persistent SBUF
        xt_sb = _persist(pp, [128, NCC, T], BF16, "xts")
        wq_sb = _persist(pp, [128, NCC, CL], BF16, "wqs")
        wk_sb = _persist(pp, [128, NCC, CL], BF16, "wks")
        wv_sb = _persist(pp, [128, NCC, CL], BF16, "wvs")
        wp_sb = _persist(pp, [128, CL // 128, C], BF16, "wps")
        bq_sb = _persist(pp, [128, 4], F32, "bqs")
        bk_sb = _persist(pp, [128, 4], F32, "bks")
        mska_sb = _persist(pp, [128, 896], BF16, "mskas")
        mskb_sb = _persist(pp, [128, 384], BF16, "mskbs")
        ident_sb = _persist(pp, [128, 128], BF16, "idents")
        wdum_sb = _persist(pp, [128, 512], BF16, "wdums")
        qt_sb = [_persist(pp, [128, T], BF16, f"qts{p}") for p in range(4)]
        kt_sb = [_persist(pp, [128, T], BF16, f"kts{p}") for p in range(4)]
        # V with a trailing ones column per head: [t-part, t-chunk, head, 64+1]
        v_sb = _persist(pp, [128, NTC, HL, DH + 1], BF16, "vsb")
        aot_sb = [
            [_persist(pp, [128, 512], BF16, f"aots{p}_{jj}") for jj in range(T // 512)]
            for p in range(4)
        ]

        for _rep in range(loop_reps):
            _emit_body(
                nc, tc, xt, wq, wk, wv, wp, bq, bk, mska, mskb, ident, out,
                xt_sb, wq_sb, wk_sb, wv_sb, wp_sb, bq_sb, bk_sb, mska_sb,
                mskb_sb, ident_sb, wdum_sb, qt_sb, kt_sb, v_sb, aot_sb,
            )
    _split_sync_waits(nc)
    return nc


def _emit_body(
    nc, tc, xt, wq, wk, wv, wp, bq, bk, mska, mskb, ident, out,
    xt_sb, wq_sb, wk_sb, wv_sb, wp_sb, bq_sb, bk_sb, mska_sb,
    mskb_sb, ident_sb, wdum_sb, qt_sb, kt_sb, v_sb, aot_sb,
):
    NCC = C // 128
    NTS = T // 512

    def _rows(w, lo, hi):
        # w[lo:hi, :] laid out [p, chunk, outcol]
        return w[lo:hi].rearrange("(c p) o -> p c o", p=128)

    # All large transfers on the sync HWDGE queue in exact need-order, so
    # the (serial) DMA-engine track processes them in the order the PE
    # stream consumes them.  Tiny transfers ride the scalar queue.
    nc.sync.dma_start(
        out=xt_sb[:, 0:4, 0:512], in_=xt[0, 0:512].rearrange("(c p) t -> p c t", p=128)
    )
    nc.sync.dma_start(
        out=wq_sb[:, :, 0:256],
        in_=wq[:, 0:256].rearrange("(c p) o -> p c o", p=128),
    )
    nc.sync.dma_start(
        out=xt_sb[:, 4:8, 0:512],
        in_=xt[0, 512:1024].rearrange("(c p) t -> p c t", p=128),
    )
    nc.sync.dma_start(
        out=wq_sb[:, :, 256:512],
        in_=wq[:, 256:512].rearrange("(c p) o -> p c o", p=128),
    )
    nc.sync.dma_start(out=wk_sb, in_=_rows(wk, 0, C))
    nc.sync.dma_start(out=wv_sb, in_=_rows(wv, 0, C))
    for j in range(1, NTS):
        nc.sync.dma_start(
            out=xt_sb[:, :, 512 * j : 512 * j + 512],
            in_=xt[j].rearrange("(c p) t -> p c t", p=128),
        )
    nc.sync.dma_start(out=wp_sb, in_=wp.rearrange("(c p) o -> p c o", p=128))
    nc.scalar.dma_start(out=bq_sb, in_=bq[:])
    nc.scalar.dma_start(out=bk_sb, in_=bk[:])
    nc.scalar.dma_start(out=mska_sb, in_=mska[:])
    nc.scalar.dma_start(out=mskb_sb, in_=mskb[:])
    nc.scalar.dma_start(out=ident_sb, in_=ident[:])
    nc.vector.memset(wdum_sb, 0.0)
    nc.vector.memset(v_sb[:, :, :, DH : DH + 1], 1.0)

    # ------------------------------------------------ pools
    with (
        tc.tile_pool(name="pmm", bufs=2, space="PSUM") as pmm,
        tc.tile_pool(name="pst", bufs=2, space="PSUM") as pstp,
        tc.tile_pool(name="pso", bufs=2, space="PSUM") as psop,
        tc.tile_pool(name="pest", bufs=10) as pest,
        tc.tile_pool(name="pstg", bufs=3) as pstg,
        tc.tile_pool(name="prec", bufs=4) as prec,
        tc.tile_pool(name="pout", bufs=5) as pout,
    ):

        def qk_strip_units(j):
            """Q^T/K^T projection for t-strip j, yielded in PE-sized
            units so attention rounds can interleave them as filler.
            Strip 0 runs all Q before all K so the first matmuls only
            depend on the wq/xt transfers."""
            t0 = 512 * j
            plan = (
                [(p, 0) for p in range(4)] + [(p, 1) for p in range(4)]
                if j == 0
                else [(p, qk) for p in range(4) for qk in range(2)]
            )
            for p, qk in plan:
                w_sb, b_sb, o_sb = (
                    (wq_sb, bq_sb, qt_sb) if qk == 0 else (wk_sb, bk_sb, kt_sb)
                )
                ps = pmm.tile([128, 512], F32, name="psqk", tag="mm")
                for cc in range(NCC):
                    nc.tensor.matmul(
                        ps,
                        lhsT=w_sb[:, cc, 128 * p : 128 * p + 128],
                        rhs=xt_sb[:, cc, t0 : t0 + 512],
                        start=(cc == 0),
                        stop=(cc == NCC - 1),
                    )
                nc.vector.tensor_scalar_add(
                    o_sb[p][:, t0 : t0 + 512], ps, b_sb[:, p : p + 1]
                )
                yield

        def v_strip_units(j):
            """V projection for the 4 t-chunks of strip j."""
            for ic in range(4 * j, 4 * j + 4):
                psv = pmm.tile([128, 512], F32, name="psv", tag="mm")
                for cc in range(NCC):
                    nc.tensor.matmul(
                        psv,
                        lhsT=xt_sb[:, cc, 128 * ic : 128 * ic + 128],
                        rhs=wv_sb[:, cc, :],
                        start=(cc == 0),
                        stop=(cc == NCC - 1),
                    )
                nc.vector.tensor_copy(
                    v_sb[:, ic, :, 0:DH],
                    psv.rearrange("p (h d) -> p h d", h=HL),
                )
                yield

        def qkv_strip_units(j):
            yield from qk_strip_units(j)
            yield from v_strip_units(j)

        def _proj_mm(j, qi, nh, ccs):
            ps3 = pmm.tile([128, 512], F32, name="ps3", tag="mm")
            for ci, cc in enumerate(ccs):
                nc.tensor.matmul(
                    ps3,
                    lhsT=aot_sb[cc][j][:, 128 * (qi % 4) : 128 * (qi % 4) + 128],
                    rhs=wp_sb[:, cc, 512 * nh : 512 * nh + 512],
                    start=(ci == 0),
                    stop=(ci == len(ccs) - 1),
                )
            return ps3

        def proj_units(j):
            """Partial output projection for the 4 t-chunks of strip j."""
            for qi in range(4 * j, 4 * j + 4):
                ostg = pout.tile([128, C], BF16, name="ostg", tag="ostg")
                for nh in range(2):
                    ps3 = _proj_mm(j, qi, nh, range(CL // 128))
                    nc.vector.tensor_copy(ostg[:, 512 * nh : 512 * nh + 512], ps3)
                    nc.sync.dma_start(
                        out=out[128 * qi : 128 * qi + 128, 512 * nh : 512 * nh + 512],
                        in_=ostg[:, 512 * nh : 512 * nh + 512],
                    )
                    yield

        lstg = {}  # (qi, nh) -> bf16 staging of the cc{0,1} half-sum

        def proj_last_part1(j):
            """First half-accumulation (pairs 0,1) of the last strip's
            projection -- runs as round-3 filler once aot[0..1][j] exist,
            adding PE work to the exp-bound closing round."""
            for qi in range(4 * j, 4 * j + 4):
                for nh in range(2):
                    ps3 = _proj_mm(j, qi, nh, (0, 1))
                    st_ = pout.tile([128, 512], BF16, name="lstg", tag="lstg", bufs=16)
                    nc.vector.tensor_copy(st_, ps3)
                    lstg[(qi, nh)] = st_
                    yield

        def proj_last_part2(j):
            """Second half (pairs 2,3) + combine + store.  Emitted in
            2-unit waves whose cc=2 matmuls (pair 2's aot is ready well
            before the tail pair's) run while the last pair normalizes
            and transposes."""
            units = [(qi, nh) for qi in range(4 * j, 4 * j + 4) for nh in range(2)]
            ostgs = {}
            for qi in range(4 * j, 4 * j + 4):
                ostgs[qi] = pout.tile([128, C], BF16, name="ostg", tag="ostg")
            for w in range(0, len(units), 2):
                wave = []
                for qi, nh in units[w : w + 2]:
                    ps3 = pmm.tile([128, 512], F32, name="ps3", tag="mm")
                    nc.tensor.matmul(
                        ps3,
                        lhsT=aot_sb[2][j][:, 128 * (qi % 4) : 128 * (qi % 4) + 128],
                        rhs=wp_sb[:, 2, 512 * nh : 512 * nh + 512],
                        start=True,
                        stop=False,
                    )
                    wave.append((qi, nh, ps3))
                    yield
                for qi, nh, ps3 in wave:
                    nc.tensor.matmul(
                        ps3,
                        lhsT=aot_sb[3][j][:, 128 * (qi % 4) : 128 * (qi % 4) + 128],
                        rhs=wp_sb[:, 3, 512 * nh : 512 * nh + 512],
                        start=False,
                        stop=True,
                    )
                    nc.vector.tensor_add(
                        ostgs[qi][:, 512 * nh : 512 * nh + 512], ps3, lstg[(qi, nh)]
                    )
                    nc.sync.dma_start(
                        out=out[128 * qi : 128 * qi + 128, 512 * nh : 512 * nh + 512],
                        in_=ostgs[qi][:, 512 * nh : 512 * nh + 512],
                    )
                    yield

        PV_LAG = 4  # plan steps (2 k-chunks each) the PV trails behind

        stages = {}  # pair -> stage tile for the pair currently in flight

        def attn(h, j, pump, tail=False):
            """Causal attention for head h over q-strip j.  The score/exp
            stream is software-pipelined PV_LAG plan-steps (2 k-chunks
            each) ahead of the flipped PV accumulation; the four
            diagonal blocks are packed into two PSUM tiles so they cost
            two exp/mask instructions instead of four; `pump` is called
            once per step to emit filler projection work that keeps PE
            busy while the Scalar engine works through the exp stream."""
            pt, off = h // 2, (h % 2) * DH
            qt_h = qt_sb[pt][off : off + DH, :]
            kt_h = kt_sb[pt][off : off + DH, :]
            q0 = 512 * j
            nk = 4 * (j + 1)
            pso = psop.tile([128, 4, PVW], F32, name="pso", tag="pso")
            # ests[ip] = (ap, shift): PV lhsT for q-sub n is
            # ap[:, 128n - shift : 128n - shift + 128]
            ests = [None] * nk
            nfull = 4 * j

            def emit_full_pair(i):
                pst2 = pstp.tile([128, 1024], F32, name="pst2", tag="st")
                for half in range(2):
                    nc.tensor.matmul(
                        pst2[:, 512 * half : 512 * half + 512],
                        lhsT=kt_h[:, 128 * (i + half) : 128 * (i + half) + 128],
                        rhs=qt_h[:, q0 : q0 + 512],
                        start=True,
                        stop=True,
                    )
                est2 = pest.tile([128, 1024], BF16, name="est2", tag="est")
                nc.scalar.activation(est2, pst2, AF.Exp, scale=SCALE)
                ests[i] = (est2[:, 0:512], 0)
                ests[i + 1] = (est2[:, 512:1024], 0)

            def emit_diag_pair(pos):
                # pos 0: diagonals m=0,1 packed at cols [0:512 | 512:896]
                # pos 1: diagonals m=2,3 packed at cols [0:256 | 256:384]
                m0 = 2 * pos
                w0 = 512 - 128 * m0          # live width of diag m0
                w1 = w0 - 128                # live width of diag m0+1
                pst2 = pstp.tile([128, 1024], F32, name="pst2", tag="st")
                nc.tensor.matmul(
                    pst2[:, 0:w0],
                    lhsT=kt_h[:, 128 * (nfull + m0) : 128 * (nfull + m0) + 128],
                    rhs=qt_h[:, q0 + 128 * m0 : q0 + 512],
                    start=True,
                    stop=True,
                )
                nc.tensor.matmul(
                    pst2[:, w0 : w0 + w1],
                    lhsT=kt_h[:, 128 * (nfull + m0 + 1) : 128 * (nfull + m0 + 1) + 128],
                    rhs=qt_h[:, q0 + 128 * (m0 + 1) : q0 + 512],
                    start=True,
                    stop=True,
                )
                est2 = pest.tile([128, 1024], BF16, name="est2", tag="est")
                nc.scalar.activation(
                    est2[:, 0 : w0 + w1], pst2[:, 0 : w0 + w1], AF.Exp, scale=SCALE
                )
                mk = mska_sb if pos == 0 else mskb_sb
                nc.vector.tensor_mul(
                    est2[:, 0 : w0 + w1], est2[:, 0 : w0 + w1], mk[:, 0 : w0 + w1]
                )
                ests[nfull + m0] = (est2[:, 0:w0], 128 * m0)
                ests[nfull + m0 + 1] = (est2[:, w0 : w0 + w1], 128 * (m0 + 1))

            plan = [("full", i) for i in range(0, nfull, 2)]
            plan += [("diag", 0), ("diag", 1)]
            nplan = len(plan)
            for s in range(nplan + PV_LAG):
                if s < nplan:
                    kind, arg = plan[s]
                    if kind == "full":
                        emit_full_pair(arg)
                    else:
                        emit_diag_pair(arg)
                    pump()
                if s >= PV_LAG:
                    # flipped PV for the two k-chunks of step s - PV_LAG:
                    # per live q-sub, one [128, 65] matmul with lhsT =
                    # the est q-sub slice.  The very first matmul carries
                    # the PSUM start flag (bank-granular lazy zeroing);
                    # everything later lands on zeroed-or-written bytes.
                    sp = s - PV_LAG
                    for ip in range(2 * sp, 2 * sp + 2):
                        a_, sh = ests[ip]
                        for n in range(max(0, ip - 4 * j), 4):
                            nc.tensor.matmul(
                                pso[:, n, 0:65],
                                lhsT=a_[:, 128 * n - sh : 128 * n - sh + 128],
                                rhs=v_sb[:, ip, h, :],
                                start=(ip == 0 and n == 0),
                                stop=(ip == nk - 1 and n == 3),
                                skip_group_check=True,
                            )
            # normalize: denominators sit in column 64 of each q-sub
            # accumulator -- one batched reciprocal, then per-sub
            # per-partition scaling into the [q, ch] staging tile
            if h % 2 == 0:
                stages[pt] = pstg.tile([128, 4, 128], BF16, name="stg", tag="stg")
            stage = stages[pt]
            rec4 = prec.tile([128, 4], F32, name="rec4", tag="rec4")
            nc.vector.reciprocal(
                rec4, pso[:, :, 64:65].rearrange("p n o -> p (n o)")
            )
            for n in range(4):
                nc.vector.tensor_scalar_mul(
                    stage[:, n, off : off + DH],
                    pso[:, n, 0:DH],
                    rec4[:, n : n + 1],
                )
            if h % 2 == 1 and not tail:
                # XBAR DMA-transpose [q, (sub, ch)] -> [ch, (sub, q)]:
                # lands the pair's attn-out in output-projection layout
                # with zero PE/DVE work
                nc.sync.dma_start_transpose(
                    aot_sb[pt][j].rearrange("p (n q) -> p n q", n=4), stage
                )

        def finalize_tail_pair(pt, j):
            """PE-transpose path for the last pair: shorter latency than
            the XBAR DMA-transpose, and PE is idle at this point."""
            stage = stages[pt]
            ptile = psop.tile([128, 512], BF16, name="ptile", tag="pso")
            for n in range(4):
                nc.tensor.matmul(
                    ptile[:, 128 * n : 128 * n + 128],
                    lhsT=stage[:, n, :],
                    rhs=ident_sb,
                    is_transpose=True,
                    start=True,
                    stop=True,
                )
            nc.vector.tensor_copy(aot_sb[pt][j], ptile)

        # j-major schedule: QKV strips and output-projection chunks are
        # threaded through the attention rounds as PE filler, weighted
        # toward the later (bigger, exp-bound) rounds.
        # p-state warmups: keep PE busy (and ramping) while the first
        # xt/wq transfers land; results are never read
        for _ in range(9):
            psw = pmm.tile([128, 512], F32, name="psw", tag="mm")
            nc.tensor.matmul(
                psw, lhsT=wdum_sb[:, 0:128], rhs=wdum_sb, start=True, stop=True
            )
        for _ in qkv_strip_units(0):
            pass
        FILLER_PLAN = {
            0: [1],
            1: [2],
            2: [3],
            3: [-1, -2, -3, "p3a"],
        }
        for j in range(NTS):
            gens = []
            n_units = 0
            for f in FILLER_PLAN[j]:
                if f == "p3a":
                    gens.append(proj_last_part1(NTS - 1))
                    n_units += 8
                elif f >= 0:
                    gens.append(qkv_strip_units(f))
                    n_units += 12
                else:
                    gens.append(proj_units(-f - 1))
                    n_units += 8
            # in the last round, hold back a few filler units to run
            # after the final attention block, covering the last pair's
            # normalize-chain latency before the output projection
            reserve = 2 if j == NTS - 1 else 0
            usable = max(1, n_units - reserve)
            pump_calls = HL * (2 * j + 2)
            quota = (usable / max(1, pump_calls - 3)) if n_units else 0.0

            def _advance():
                while gens:
                    try:
                        next(gens[0])
                        return
                    except StopIteration:
                        gens.pop(0)

            state = {"cnt": 0, "used": 0, "credit": 0.0}

            def pump():
                state["cnt"] += 1
                # let the strip's xt DMAs land before filler matmuls;
                # stop at the usable budget so `reserve` units remain
                # for the post-round drain
                if state["cnt"] < 3:
                    return
                state["credit"] += quota
                while state["credit"] >= 1.0 and state["used"] < usable:
                    state["credit"] -= 1.0
                    state["used"] += 1
                    _advance()

            for h in range(HL):
                attn(h, j, pump, tail=(j == NTS - 1 and h == HL - 1))
            for g in gens:
                for _ in g:
                    pass
            if j == NTS - 1:
                p2 = proj_last_part2(NTS - 1)
                next(p2)
                next(p2)
                finalize_tail_pair(HL // 2 - 1, j)
                for _ in p2:
                    pass


_PROGRAM = None


def _get_program():
    global _PROGRAM
    if _PROGRAM is None:
        _PROGRAM = build_program()
    return _PROGRAM


def _make_packed_masks():
    """Causal 0/1 masks for the packed diagonal-pair est tiles.
    mska[i, c] allows (c >= i) for c < 512 (diag m=0) and (c-512 >= i)
    for c >= 512 (diag m=1); mskb is the same at half scale (m=2, 3)."""
    i = np.arange(128)[:, None]
    ca = np.arange(896)[None, :]
    mska = np.where(ca < 512, ca >= i, (ca - 512) >= i)
    cb = np.arange(384)[None, :]
    mskb = np.where(cb < 256, cb >= i, (cb - 256) >= i)
    bf = ml_dtypes.bfloat16
    return mska.astype(bf), mskb.astype(bf)


def make_in_maps(x, W_qkv, b_qkv, W_proj):
    """Shard the full inputs into the 8 per-core input maps."""
    x = np.asarray(x, dtype=np.float32)
    W_qkv = np.asarray(W_qkv, dtype=np.float32)
    b_qkv = np.asarray(b_qkv, dtype=np.float32)
    W_proj = np.asarray(W_proj, dtype=np.float32)
    bf = ml_dtypes.bfloat16
    mska, mskb = _make_packed_masks()
    ident = np.eye(128, dtype=bf)
    in_maps = []
    for core in range(NCORES):
        b, g = core // 2, core % 2
        cs = slice(CL * g, CL * g + CL)
        xt = np.ascontiguousarray(
            x[b].T.reshape(C, T // 512, 512).transpose(1, 0, 2)
        ).astype(bf)
        wq_s = np.ascontiguousarray(W_qkv[:, CL * g : CL * g + CL]).astype(bf)
        wk_s = np.ascontiguousarray(W_qkv[:, C + CL * g : C + CL * g + CL]).astype(bf)
        wv_s = np.ascontiguousarray(
            W_qkv[:, 2 * C + CL * g : 2 * C + CL * g + CL]
        ).astype(bf)
        wp_s = np.ascontiguousarray(W_proj[CL * g : CL * g + CL, :]).astype(bf)
        bq_s = np.ascontiguousarray(b_qkv[cs].reshape(4, 128).T)
        bk_s = np.ascontiguousarray(b_qkv[C + CL * g : C + CL * g + CL].reshape(4, 128).T)
        in_maps.append(
            {
                "xt": xt,
                "wq": wq_s,
                "wk": wk_s,
                "wv": wv_s,
                "wp": wp_s,
                "bq": bq_s,
                "bk": bk_s,
                "mska": mska,
                "mskb": mskb,
                "ident": ident,
            }
        )
    return in_maps


def gather_output(results, b_qkv, W_proj, b_proj):
    """Sum the per-core partial outputs and fold in the host-side biases."""
    b_qkv = np.asarray(b_qkv, dtype=np.float32)
    W_proj = np.asarray(W_proj, dtype=np.float32)
    b_proj = np.asarray(b_proj, dtype=np.float32)
    bv = b_qkv[2 * C : 3 * C]
    extra = (bv @ W_proj + b_proj).astype(np.float32)
    out = np.empty((B, T, C), dtype=np.float32)
    for b in range(B):
        out[b] = (
            np.asarray(results[2 * b]["out"], dtype=np.float32)
            + np.asarray(results[2 * b + 1]["out"], dtype=np.float32)
            + extra
        )
    return out


def kernel(x, W_qkv, b_qkv, W_proj, b_proj):
    nc = _get_program()
    in_maps = make_in_maps(x, W_qkv, b_qkv, W_proj)
    res = run_bass_kernel_spmd(nc, in_maps, list(range(NCORES)))
    return gather_output(res.results, b_qkv, W_proj, b_proj)
